# revision 28
# baseline (speedup 1.0000x reference)
"""Tensor-parallel FlashLlamaAttention kernel for 8 Trainium2 NeuronCores.

Sharding: each core owns 4 query heads (512 proj dims) and 1 kv head
(128 dims). Per-core device program computes qkv projection (+RoPE),
causal GQA attention and its o_proj partial product; the 8 partial
[2048, 4096] outputs are summed on the host (replaces the all-reduce).

v3: fully-fused single PE stream in bf16.
 - all matmul operands bf16 (1 cyc/row, same as f32r, but half the DMA
   and SBUF traffic); PSUM stays f32; host pre-casts inputs, output is
   written bf16 and summed in f32 on the host.
 - projection chunks, attention groups and o_proj blocks are emitted
   interleaved so the PE never idles across phase boundaries (idle gaps
   also reset the PE DVFS ramp).
 - k and v are projected in each chunk's FIRST pass so the next
   segment's attention unblocks a full pass earlier.
 - RoPE rotate-half runs as a partition-crossing bf16 DMA instead of a
   PE permutation matmul.
 - V is projected directly in [token, dim] layout by using the hidden
   chunk as the matmul stationary, killing the PE transposes.
 - softmax denominator is accumulated broadcast across partitions via a
   ones[128,128] stationary and inverted with the fast approximate
   reciprocal (the exact DVE reciprocal costs 3.3us per row-tile and
   serialized the in-order DVE queue).
"""
import sys

sys.path.insert(0, "/opt/trn_rl_repo")

from contextlib import ExitStack

import numpy as np
import ml_dtypes

import concourse.bass as bass
import concourse.bacc as bacc
import concourse.mybir as mybir
import concourse.tile as tile
from concourse.bass_utils import run_bass_kernel_spmd
from concourse.masks import make_identity

F32 = mybir.dt.float32
BF16 = mybir.dt.bfloat16
EXP = mybir.ActivationFunctionType.Exp

P = 128          # partitions / head dim
T = 2048         # total tokens (B * S)
S = 1024         # seq len per batch
B = 2
HD = 4096        # hidden dim
NHL = 4          # local query heads per core
DQKV = NHL * P + P + P  # 768 local projection dims (4q + k + v)
CH = 512         # token chunk for projection
KO = HD // P     # 32 contraction chunks
SM = float(P) ** -0.5

N_CORES = 8

_SENT = object()


def build_nc():
    nc = bacc.Bacc("TRN2", target_bir_lowering=False, debug=False,
                   num_devices=N_CORES)

    hiddenT = nc.dram_tensor("hiddenT", [HD, T], BF16, kind="ExternalInput").ap()
    # wqB holds the six projection blocks (k, q0..q3, v) already in SBUF
    # image layout [p, g, c] so weight DMAs move 2KB-contiguous lines
    wqB = nc.dram_tensor("wqB", [6, P, KO * P], BF16, kind="ExternalInput").ap()
    woT = nc.dram_tensor("woT", [NHL * P, HD], BF16, kind="ExternalInput").ap()
    cosF = nc.dram_tensor("cosF", [P, T], BF16, kind="ExternalInput").ap()
    sinF = nc.dram_tensor("sinF", [P, T], BF16, kind="ExternalInput").ap()
    out = nc.dram_tensor("out", [T, HD], BF16, kind="ExternalOutput").ap()

    with tile.TileContext(nc) as tc, ExitStack() as stack:
        const = stack.enter_context(tc.tile_pool(name="const", bufs=1))
        ident = const.tile([P, P], F32)
        make_identity(nc, ident[:])
        ones_sb = const.tile([P, P], BF16)
        nc.vector.memset(ones_sb[:], 1.0)
        # causal corner mask: keep (q=f) >= (k=p)
        mask = const.tile([P, P], BF16)
        nc.gpsimd.memset(mask[:], 1.0)
        nc.gpsimd.affine_select(
            out=mask[:], in_=mask[:], compare_op=mybir.AluOpType.is_ge,
            fill=0.0, base=0, pattern=[[1, P]], channel_multiplier=-1)

        w1 = stack.enter_context(tc.tile_pool(name="w1", bufs=1))
        wqkv_sb = w1.tile([P, 6, KO, P], BF16)
        cs = stack.enter_context(tc.tile_pool(name="cs", bufs=1))
        cos_sb = cs.tile([P, T], BF16)
        sin_sb = cs.tile([P, T], BF16)
        qk = stack.enter_context(tc.tile_pool(name="qk", bufs=1))
        qT = [qk.tile([P, T], BF16, tag=f"qT{h}", name=f"qT{h}")
              for h in range(NHL)]
        kT = qk.tile([P, T], BF16, tag="kT", name="kT")
        vpool = stack.enter_context(tc.tile_pool(name="vp", bufs=1))
        v_sb = vpool.tile([P, T // P, P], BF16, tag="v_sb", name="v_sb")
        atn_pool = stack.enter_context(tc.tile_pool(name="atn", bufs=1))
        ATn = [atn_pool.tile([P, T], BF16, tag=f"ATn{h}", name=f"ATn{h}")
               for h in range(NHL)]
        w2 = stack.enter_context(tc.tile_pool(name="w2", bufs=1))
        wo_sb = w2.tile([P, NHL, HD], BF16)
        pexp_pool = stack.enter_context(tc.tile_pool(name="pexp", bufs=6))
        rden_pool = stack.enter_context(tc.tile_pool(name="rden", bufs=3))

        # weight loads on the Act DMA ring in quarter-block granularity,
        # ordered by first use: k/v (chunk pass 1), rope tables, then q0..q3
        def wsub(b, i, eng=None):
            (eng or nc.scalar).dma_start(
                wqkv_sb[:, b, 8 * i:8 * (i + 1), :],
                wqB[b, :, 1024 * i:1024 * (i + 1)])
        for i in range(4):
            wsub(0, i)
            wsub(5, i)
        nc.scalar.dma_start(cos_sb[:], cosF[:])
        nc.scalar.dma_start(sin_sb[:], sinF[:])
        for j in (1, 2, 3, 4):
            for i in range(4):
                wsub(j, i)
        # wo is needed only from segment C; its 8 half-slices are issued at
        # pass boundaries inside chunks 1-3 so they never block the rope DMAs
        woT_r = woT.rearrange("(a p) o -> p a o", p=P)
        wo_side = [
            (lambda g=g: nc.sync.dma_start(wo_sb[:, g, :], woT_r[:, g, :]))
            for g in range(NHL)
        ]

        # attention/transpose PSUM pools live for the whole run
        st_psum = stack.enter_context(
            tc.tile_pool(name="stp", bufs=2, space="PSUM"))
        at_psum = stack.enter_context(
            tc.tile_pool(name="atp", bufs=2, space="PSUM"))
        den_psum = stack.enter_context(
            tc.tile_pool(name="dnp", bufs=2, space="PSUM"))

        xt_stack = ExitStack()
        xt_pool = xt_stack.enter_context(tc.tile_pool(name="xt", bufs=32))
        rot_pool = xt_stack.enter_context(tc.tile_pool(name="rot", bufs=3))
        rt_pool = xt_stack.enter_context(tc.tile_pool(name="rt", bufs=3))
        vt_pool = xt_stack.enter_context(tc.tile_pool(name="vt", bufs=2))
        qr_pool = xt_stack.enter_context(tc.tile_pool(name="qr", bufs=3))

        def load_xt(c):
            ts = slice(CH * c, CH * (c + 1))
            xts = []
            for ho in range(KO):
                t = xt_pool.tile([P, CH], BF16, tag="xt",
                                 name=f"xt{c}_{ho}")
                nc.sync.dma_start(t[:], hiddenT[P * ho:P * (ho + 1), ts])
                xts.append(t)
            return xts

        xt_next = {}

        def prefetch(c):
            return lambda: xt_next.__setitem__(c, load_xt(c))

        def proj_chunk(c, pp, side=(), pre=None):
            """qkv projection + rope for tokens [512c, 512c+512).

            k and v are produced in the FIRST pass so the next segment's
            attention (which needs kT/v_sb before qT) unblocks a full pass
            earlier. `side` DMA thunks are fired at pass boundaries.
            """
            side = list(side)
            ts = slice(CH * c, CH * (c + 1))
            xts = xt_next.pop(c, None) or load_xt(c)
            if pre is not None:
                pre()

            def qk_evac1(idx, ps):
                # dst = ps*cosF + rothalf(ps*sinG); sinG = sinF[rot(d)]
                # so the partition-crossing move runs as a bf16 DMA.
                # One Act copy reads the PSUM bank (releasing it for the
                # next pass in ~640ns instead of 2x680ns serial DVE muls);
                # the muls then run 2X-bf16 off SBUF.
                dst = qT[idx] if idx < NHL else kT
                qraw = qr_pool.tile([P, CH], BF16, tag="qraw")
                nc.scalar.activation(qraw[:], ps[:],
                                     mybir.ActivationFunctionType.Copy)
                nc.vector.tensor_mul(out=dst[:, ts], in0=qraw[:],
                                     in1=cos_sb[:, ts])
                qs = rot_pool.tile([P, CH], BF16, tag="qs")
                nc.vector.tensor_mul(out=qs[:], in0=qraw[:],
                                     in1=sin_sb[:, ts])
                rt = rt_pool.tile([P, CH], BF16, tag="rt")
                nc.scalar.dma_start(rt[64:128, :], qs[0:64, :])
                nc.scalar.dma_start(rt[0:64, :], qs[64:128, :])
                return dst, rt

            def qk_evac2(st):
                dst, rt = st
                nc.vector.tensor_add(out=dst[:, ts], in0=dst[:, ts],
                                     in1=rt[:])

            def vt_evac1(psV):
                vt = vt_pool.tile([P, CH], F32, tag="vt")
                nc.scalar.activation(vt[:], psV[:],
                                     mybir.ActivationFunctionType.Copy)
                return vt

            def vt_evac2(vt):
                # transpose [dim, tok] -> [tok, dim] on the PE via the den
                # psum pool (idle slot between attention groups)
                trp = den_psum.tile([P, CH], F32, tag="den", name=f"trp{c}")
                for s4 in range(4):
                    nc.tensor.transpose(trp[:, P * s4:P * (s4 + 1)],
                                        vt[:, P * s4:P * (s4 + 1)], ident[:])
                nc.vector.tensor_copy(v_sb[:, 4 * c:4 * (c + 1), :], trp[:])

            def kq_pass(idxs, banks, mid=None):
                # one output group per bank, interleaved per-ho across banks
                for ho in range(KO):
                    if ho == 6 and mid is not None:
                        mid()
                    for bi, ps in zip(idxs, banks):
                        nc.tensor.matmul(ps[:], wqkv_sb[:, bi, ho, :],
                                         xts[ho][:], start=(ho == 0),
                                         stop=(ho == KO - 1))
                        yield
                sts = [qk_evac1(0 if bi == 1 else (NHL if bi == 0 else bi - 1),
                                ps)
                       for bi, ps in zip(idxs, banks) if bi != 5]
                vts = [vt_evac1(ps)
                       for bi, ps in zip(idxs, banks) if bi == 5]
                for st_ in sts:
                    qk_evac2(st_)
                return_vals.append(vts)

            def boundary():
                if side:
                    side.pop(0)()

            def bank(nm):
                return pp.tile([P, CH], F32, tag="pp", name=nm)

            return_vals = []
            # pass 1: k + v (blocks 0, 5)
            yield from kq_pass((0, 5), [bank("ppk"), bank("ppv")])
            vt = return_vals[-1][0]
            boundary()
            # pass 2: q0 + q1 (blocks 1, 2); v transpose slots in mid-pass
            yield from kq_pass((1, 2), [bank("ppq0"), bank("ppq1")],
                               mid=lambda: vt_evac2(vt))
            boundary()
            # pass 3: q2 + q3 (blocks 3, 4)
            yield from kq_pass((3, 4), [bank("ppq2"), bank("ppq3")])
            boundary()

        def chain(*gens):
            for g in gens:
                yield from g

        def interleave(pg, ag, ratio):
            """1 attention yield : `ratio` proj yields; drain both"""
            done_p = done_a = False
            while not (done_p and done_a):
                if not done_a and next(ag, _SENT) is _SENT:
                    done_a = True
                for _ in range(ratio if not done_a else 1 << 30):
                    if next(pg, _SENT) is _SENT:
                        done_p = True
                        break
                if done_p and not done_a:
                    for _ in ag:
                        pass
                    done_a = True


        def attn_group(b, h, qt):
            """causal attention for one (batch, head, 512-q-block)"""
            qTb = qT[h][:, S * b:S * (b + 1)]
            kTb = kT[:, S * b:S * (b + 1)]
            at_ps = at_psum.tile([P, CH], F32, tag="at")
            den_ps = den_psum.tile([P, CH], F32, tag="den")
            nk = 4 * qt + 4
            pend = []

            def flush():
                a, qo, px = pend.pop(0)
                nc.tensor.matmul(at_ps[:, qo:], v_sb[:, (S // P) * b + a, :],
                                 px[:, qo:], start=(a == 0),
                                 stop=(a == nk - 1))
                nc.tensor.matmul(den_ps[:, qo:], ones_sb[:], px[:, qo:],
                                 start=(a == 0), stop=(a == nk - 1))

            for a in range(nk):
                qoff = max(0, P * a - CH * qt)
                st = st_psum.tile([P, CH], F32, tag="st")
                nc.tensor.matmul(st[:, qoff:], kTb[:, P * a:P * (a + 1)],
                                 qTb[:, CH * qt + qoff:CH * (qt + 1)],
                                 start=True, stop=True)
                px = pexp_pool.tile([P, CH], BF16, tag="pexp")
                nc.scalar.activation(px[:, qoff:], st[:, qoff:], EXP,
                                     scale=SM)
                if P * a >= CH * qt:
                    nc.vector.tensor_mul(out=px[:, qoff:qoff + P],
                                         in0=px[:, qoff:qoff + P],
                                         in1=mask[:])
                pend.append((a, qoff, px))
                if len(pend) == 3:
                    flush()
                yield
            while pend:
                flush()
            rden = rden_pool.tile([P, CH], F32, tag="rden")
            nc.vector.reciprocal_approx_fast(out=rden[:], in_=den_ps[:])
            nc.vector.tensor_mul(
                out=ATn[h][:, S * b + CH * qt:S * b + CH * (qt + 1)],
                in0=at_ps[:], in1=rden[:])
            yield

        def attn_seq(b, qts):
            for qt in qts:
                for h in range(NHL):
                    yield from attn_group(b, h, qt)

        with tc.tile_pool(name="pp", bufs=2, space="PSUM") as pp:
            # chunk 0 alone (nothing else is ready); remaining weight
            # blocks stream in behind its hidden-state loads
            # with bufs=32 the next chunk's xt[j] reuses this chunk's
            # xt[j] (freed progressively through pass 3), so prefetching at
            # the first pass boundary streams supply one full pass ahead
            for _ in proj_chunk(0, pp, side=[prefetch(1)]):
                pass
            # chunk 1 (192y) x b0 qt0 attention (20y)
            interleave(proj_chunk(1, pp, side=[prefetch(2), wo_side[0],
                                               wo_side[1]]),
                       attn_seq(0, [0]), 9)
            # chunks 2+3 (384y) x b0 qt1 attention (36y)
            interleave(chain(proj_chunk(2, pp, side=[prefetch(3),
                                                     wo_side[2],
                                                     wo_side[3]]),
                             proj_chunk(3, pp)),
                       attn_seq(0, [1]), 10)
        xt_stack.close()

        with (
            tc.tile_pool(name="ob", bufs=2) as ob_pool,
            tc.tile_pool(name="opp", bufs=2, space="PSUM") as opp,
        ):
            def oproj_block(t16):
                ob = ob_pool.tile([P, HD], BF16, tag="ob")
                for ot in range(HD // CH):
                    ps = opp.tile([P, CH], F32, tag="op")
                    for j in range(NHL):
                        nc.tensor.matmul(ps[:],
                                         ATn[j][:, P * t16:P * (t16 + 1)],
                                         wo_sb[:, j, CH * ot:CH * (ot + 1)],
                                         start=(j == 0),
                                         stop=(j == NHL - 1))
                        yield
                    nc.any.tensor_copy(ob[:, CH * ot:CH * (ot + 1)], ps[:])
                    if t16 == T // P - 1:
                        nc.sync.dma_start(
                            out[P * t16:P * (t16 + 1),
                                CH * ot:CH * (ot + 1)],
                            ob[:, CH * ot:CH * (ot + 1)])
                    elif ot % 2 == 1:
                        # write out per 1024-col pair so the final block's
                        # store overlaps its own compute (shorter tail)
                        nc.sync.dma_start(
                            out[P * t16:P * (t16 + 1),
                                CH * (ot - 1):CH * (ot + 1)],
                            ob[:, CH * (ot - 1):CH * (ot + 1)])

            # b1 attention x o_proj; blocks 0-7 (b0) ready at entry,
            # 8-11 after b1 qt0 evacs (attn yield 20), 12-15 at the end
            og = chain(*[oproj_block(t16) for t16 in range(T // P)])
            consumed = 0
            ready = 8
            ay = 0
            for _ in attn_seq(1, [0, 1]):
                ay += 1
                if ay >= 20:
                    ready = max(ready, 12)
                cap = ready * 32
                pulled = 0
                while consumed < cap and pulled < 8:
                    if next(og, _SENT) is _SENT:
                        break
                    consumed += 1
                    pulled += 1
            while next(og, _SENT) is not _SENT:
                pass

    nc.compile()
    return nc


_NC = None


def _get_nc():
    global _NC
    if _NC is None:
        _NC = build_nc()
    return _NC


def make_in_maps(hidden_states, cos, sin, wq, wk, wv, wo):
    bf = ml_dtypes.bfloat16
    hs = np.asarray(hidden_states, np.float32)
    HT = np.ascontiguousarray(hs.T).astype(bf)
    cosT = np.asarray(cos, np.float32).T
    sinT = np.asarray(sin, np.float32).T
    cosF = np.ascontiguousarray(np.concatenate([cosT, cosT], 0)).astype(bf)
    sinF = np.ascontiguousarray(np.concatenate([sinT, -sinT], 0)).astype(bf)
    wq = np.asarray(wq, np.float32)
    wk = np.asarray(wk, np.float32)
    wv = np.asarray(wv, np.float32)
    wo = np.asarray(wo, np.float32)

    def sb_img(Wb):
        # [p, g, c] SBUF image: sb[p, g*128+c] = Wb[c, g*128+p]
        A = np.ascontiguousarray(Wb.T).reshape(KO, P, P)
        return A.transpose(1, 0, 2).reshape(P, KO * P)

    in_maps = []
    for c in range(N_CORES):
        wq_c = wq[NHL * P * c:NHL * P * (c + 1)]
        wk_c = wk[P * c:P * (c + 1)]
        wv_c = wv[P * c:P * (c + 1)]
        blocks = [wk_c] + [wq_c[P * j:P * (j + 1)] for j in range(NHL)] \
            + [wv_c]
        wqB = np.ascontiguousarray(
            np.stack([sb_img(b) for b in blocks], 0)).astype(bf)
        woT = np.ascontiguousarray(
            wo[:, NHL * P * c:NHL * P * (c + 1)].T).astype(bf)
        in_maps.append(dict(hiddenT=HT, wqB=wqB, woT=woT,
                            cosF=cosF, sinF=sinF))
    return in_maps


def kernel(hidden_states, cos, sin, wq, wk, wv, wo, batch, seq_len):
    assert int(batch) == B and int(seq_len) == S
    nc = _get_nc()
    in_maps = make_in_maps(hidden_states, cos, sin, wq, wk, wv, wo)
    res = run_bass_kernel_spmd(nc, in_maps, core_ids=list(range(N_CORES)))
    acc = res.results[0]["out"].astype(np.float32)
    for c in range(1, N_CORES):
        acc += res.results[c]["out"].astype(np.float32)
    return acc


# revision 29
# speedup vs baseline: 1.0330x; 1.0330x over previous
"""Tensor-parallel FlashLlamaAttention kernel for 8 Trainium2 NeuronCores.

Sharding: each core owns 4 query heads (512 proj dims) and 1 kv head
(128 dims). Per-core device program computes qkv projection (+RoPE),
causal GQA attention and its o_proj partial product; the 8 partial
[2048, 4096] outputs are summed on the host (replaces the all-reduce).

v3: fully-fused single PE stream in bf16.
 - all matmul operands bf16 (1 cyc/row, same as f32r, but half the DMA
   and SBUF traffic); PSUM stays f32; host pre-casts inputs, output is
   written bf16 and summed in f32 on the host.
 - projection chunks, attention groups and o_proj blocks are emitted
   interleaved so the PE never idles across phase boundaries (idle gaps
   also reset the PE DVFS ramp).
 - k and v are projected in each chunk's FIRST pass so the next
   segment's attention unblocks a full pass earlier.
 - RoPE rotate-half runs as a partition-crossing bf16 DMA instead of a
   PE permutation matmul.
 - V is projected directly in [token, dim] layout by using the hidden
   chunk as the matmul stationary, killing the PE transposes.
 - softmax denominator is accumulated broadcast across partitions via a
   ones[128,128] stationary and inverted with the fast approximate
   reciprocal (the exact DVE reciprocal costs 3.3us per row-tile and
   serialized the in-order DVE queue).
"""
import sys

sys.path.insert(0, "/opt/trn_rl_repo")

from contextlib import ExitStack

import numpy as np
import ml_dtypes

import concourse.bass as bass
import concourse.bacc as bacc
import concourse.mybir as mybir
import concourse.tile as tile
from concourse.bass_utils import run_bass_kernel_spmd
from concourse.masks import make_identity

F32 = mybir.dt.float32
BF16 = mybir.dt.bfloat16
EXP = mybir.ActivationFunctionType.Exp

P = 128          # partitions / head dim
T = 2048         # total tokens (B * S)
S = 1024         # seq len per batch
B = 2
HD = 4096        # hidden dim
NHL = 4          # local query heads per core
DQKV = NHL * P + P + P  # 768 local projection dims (4q + k + v)
CH = 512         # token chunk for projection
KO = HD // P     # 32 contraction chunks
SM = float(P) ** -0.5

N_CORES = 8

_SENT = object()


def build_nc():
    nc = bacc.Bacc("TRN2", target_bir_lowering=False, debug=False,
                   num_devices=N_CORES)

    hiddenT = nc.dram_tensor("hiddenT", [HD, T], BF16, kind="ExternalInput").ap()
    # wqB holds the six projection blocks (k, q0..q3, v) already in SBUF
    # image layout [p, g, c] so weight DMAs move 2KB-contiguous lines
    wqB = nc.dram_tensor("wqB", [6, P, KO * P], BF16, kind="ExternalInput").ap()
    woT = nc.dram_tensor("woT", [NHL * P, HD], BF16, kind="ExternalInput").ap()
    cosF = nc.dram_tensor("cosF", [P, T], BF16, kind="ExternalInput").ap()
    sinF = nc.dram_tensor("sinF", [P, T], BF16, kind="ExternalInput").ap()
    out = nc.dram_tensor("out", [T, HD], BF16, kind="ExternalOutput").ap()

    with tile.TileContext(nc) as tc, ExitStack() as stack:
        const = stack.enter_context(tc.tile_pool(name="const", bufs=1))
        ident = const.tile([P, P], F32)
        make_identity(nc, ident[:])
        ones_sb = const.tile([P, P], BF16)
        nc.vector.memset(ones_sb[:], 1.0)
        # causal corner mask: keep (q=f) >= (k=p)
        mask = const.tile([P, P], BF16)
        nc.gpsimd.memset(mask[:], 1.0)
        nc.gpsimd.affine_select(
            out=mask[:], in_=mask[:], compare_op=mybir.AluOpType.is_ge,
            fill=0.0, base=0, pattern=[[1, P]], channel_multiplier=-1)

        w1 = stack.enter_context(tc.tile_pool(name="w1", bufs=1))
        wqkv_sb = w1.tile([P, 6, KO, P], BF16)
        cs = stack.enter_context(tc.tile_pool(name="cs", bufs=1))
        cos_sb = cs.tile([P, T], BF16)
        sin_sb = cs.tile([P, T], BF16)
        qk = stack.enter_context(tc.tile_pool(name="qk", bufs=1))
        qT = [qk.tile([P, T], BF16, tag=f"qT{h}", name=f"qT{h}")
              for h in range(NHL)]
        kT = qk.tile([P, T], BF16, tag="kT", name="kT")
        vpool = stack.enter_context(tc.tile_pool(name="vp", bufs=1))
        v_sb = vpool.tile([P, T // P, P], BF16, tag="v_sb", name="v_sb")
        atn_pool = stack.enter_context(tc.tile_pool(name="atn", bufs=1))
        ATn = [atn_pool.tile([P, T], BF16, tag=f"ATn{h}", name=f"ATn{h}")
               for h in range(NHL)]
        w2 = stack.enter_context(tc.tile_pool(name="w2", bufs=1))
        wo_sb = w2.tile([P, NHL, HD], BF16)
        pexp_pool = stack.enter_context(tc.tile_pool(name="pexp", bufs=6))
        rden_pool = stack.enter_context(tc.tile_pool(name="rden", bufs=3))

        # weight loads on the Act DMA ring in quarter-block granularity,
        # ordered by first use: k/v (chunk pass 1), rope tables, then q0..q3
        def wsub(b, i, eng=None):
            (eng or nc.scalar).dma_start(
                wqkv_sb[:, b, 8 * i:8 * (i + 1), :],
                wqB[b, :, 1024 * i:1024 * (i + 1)])
        for i in range(4):
            wsub(0, i)
            wsub(5, i)
        nc.scalar.dma_start(cos_sb[:], cosF[:])
        nc.scalar.dma_start(sin_sb[:], sinF[:])
        for j in (1, 2, 3, 4):
            for i in range(4):
                wsub(j, i)
        # wo is needed only from segment C; its 8 half-slices are issued at
        # pass boundaries inside chunks 1-3 so they never block the rope DMAs
        woT_r = woT.rearrange("(a p) o -> p a o", p=P)
        wo_side = [
            (lambda g=g: nc.sync.dma_start(wo_sb[:, g, :], woT_r[:, g, :]))
            for g in range(NHL)
        ]

        # attention/transpose PSUM pools live for the whole run
        st_psum = stack.enter_context(
            tc.tile_pool(name="stp", bufs=2, space="PSUM"))
        at_psum = stack.enter_context(
            tc.tile_pool(name="atp", bufs=2, space="PSUM"))
        den_psum = stack.enter_context(
            tc.tile_pool(name="dnp", bufs=2, space="PSUM"))

        xt_stack = ExitStack()
        xt_pool = xt_stack.enter_context(tc.tile_pool(name="xt", bufs=32))
        rot_pool = xt_stack.enter_context(tc.tile_pool(name="rot", bufs=3))
        rt_pool = xt_stack.enter_context(tc.tile_pool(name="rt", bufs=3))
        vt_pool = xt_stack.enter_context(tc.tile_pool(name="vt", bufs=2))
        qr_pool = xt_stack.enter_context(tc.tile_pool(name="qr", bufs=3))

        def load_xt(c):
            ts = slice(CH * c, CH * (c + 1))
            xts = []
            for ho in range(KO):
                t = xt_pool.tile([P, CH], BF16, tag="xt",
                                 name=f"xt{c}_{ho}")
                nc.sync.dma_start(t[:], hiddenT[P * ho:P * (ho + 1), ts])
                xts.append(t)
            return xts

        xt_next = {}

        def prefetch(c):
            return lambda: xt_next.__setitem__(c, load_xt(c))

        def proj_chunk(c, pp, side=(), pre=None):
            """qkv projection + rope for tokens [512c, 512c+512).

            k and v are produced in the FIRST pass so the next segment's
            attention (which needs kT/v_sb before qT) unblocks a full pass
            earlier. `side` DMA thunks are fired at pass boundaries.
            """
            side = list(side)
            ts = slice(CH * c, CH * (c + 1))
            xts = xt_next.pop(c, None) or load_xt(c)
            if pre is not None:
                pre()

            def qk_evac1(idx, ps):
                # dst = ps*cosF + rothalf(ps*sinG); sinG = sinF[rot(d)]
                # so the partition-crossing move runs as a bf16 DMA
                dst = qT[idx] if idx < NHL else kT
                nc.vector.tensor_mul(out=dst[:, ts], in0=ps[:],
                                     in1=cos_sb[:, ts])
                qs = rot_pool.tile([P, CH], BF16, tag="qs")
                nc.vector.tensor_mul(out=qs[:], in0=ps[:],
                                     in1=sin_sb[:, ts])
                rt = rt_pool.tile([P, CH], BF16, tag="rt")
                nc.scalar.dma_start(rt[64:128, :], qs[0:64, :])
                nc.scalar.dma_start(rt[0:64, :], qs[64:128, :])
                return dst, rt

            def qk_evac2(st):
                dst, rt = st
                nc.vector.tensor_add(out=dst[:, ts], in0=dst[:, ts],
                                     in1=rt[:])

            def vt_evac1(psV):
                vt = vt_pool.tile([P, CH], F32, tag="vt")
                nc.vector.tensor_copy(vt[:], psV[:])
                return vt

            def vt_evac2(vt):
                # transpose [dim, tok] -> [tok, dim] on the PE via the den
                # psum pool (idle slot between attention groups)
                trp = den_psum.tile([P, CH], F32, tag="den", name=f"trp{c}")
                for s4 in range(4):
                    nc.tensor.transpose(trp[:, P * s4:P * (s4 + 1)],
                                        vt[:, P * s4:P * (s4 + 1)], ident[:])
                nc.vector.tensor_copy(v_sb[:, 4 * c:4 * (c + 1), :], trp[:])

            def kq_pass(idxs, banks, mid=None):
                # one output group per bank, interleaved per-ho across banks
                for ho in range(KO):
                    if ho == 6 and mid is not None:
                        mid()
                    for bi, ps in zip(idxs, banks):
                        nc.tensor.matmul(ps[:], wqkv_sb[:, bi, ho, :],
                                         xts[ho][:], start=(ho == 0),
                                         stop=(ho == KO - 1))
                        yield
                sts = [qk_evac1(0 if bi == 1 else (NHL if bi == 0 else bi - 1),
                                ps)
                       for bi, ps in zip(idxs, banks) if bi != 5]
                vts = [vt_evac1(ps)
                       for bi, ps in zip(idxs, banks) if bi == 5]
                for st_ in sts:
                    qk_evac2(st_)
                return_vals.append(vts)

            def boundary():
                if side:
                    side.pop(0)()

            def bank(nm):
                return pp.tile([P, CH], F32, tag="pp", name=nm)

            return_vals = []
            # pass 1: k + v (blocks 0, 5)
            yield from kq_pass((0, 5), [bank("ppk"), bank("ppv")])
            vt = return_vals[-1][0]
            boundary()
            # pass 2: q0 + q1 (blocks 1, 2); v transpose slots in mid-pass
            yield from kq_pass((1, 2), [bank("ppq0"), bank("ppq1")],
                               mid=lambda: vt_evac2(vt))
            boundary()
            # pass 3: q2 + q3 (blocks 3, 4)
            yield from kq_pass((3, 4), [bank("ppq2"), bank("ppq3")])
            boundary()

        def chain(*gens):
            for g in gens:
                yield from g

        def interleave(pg, ag, ratio):
            """1 attention yield : `ratio` proj yields; drain both"""
            done_p = done_a = False
            while not (done_p and done_a):
                if not done_a and next(ag, _SENT) is _SENT:
                    done_a = True
                for _ in range(ratio if not done_a else 1 << 30):
                    if next(pg, _SENT) is _SENT:
                        done_p = True
                        break
                if done_p and not done_a:
                    for _ in ag:
                        pass
                    done_a = True


        def attn_group(b, h, qt):
            """causal attention for one (batch, head, 512-q-block)"""
            qTb = qT[h][:, S * b:S * (b + 1)]
            kTb = kT[:, S * b:S * (b + 1)]
            at_ps = at_psum.tile([P, CH], F32, tag="at")
            den_ps = den_psum.tile([P, CH], F32, tag="den")
            nk = 4 * qt + 4
            pend = []

            def flush():
                a, qo, px = pend.pop(0)
                nc.tensor.matmul(at_ps[:, qo:], v_sb[:, (S // P) * b + a, :],
                                 px[:, qo:], start=(a == 0),
                                 stop=(a == nk - 1))
                nc.tensor.matmul(den_ps[:, qo:], ones_sb[:], px[:, qo:],
                                 start=(a == 0), stop=(a == nk - 1))

            for a in range(nk):
                qoff = max(0, P * a - CH * qt)
                st = st_psum.tile([P, CH], F32, tag="st")
                nc.tensor.matmul(st[:, qoff:], kTb[:, P * a:P * (a + 1)],
                                 qTb[:, CH * qt + qoff:CH * (qt + 1)],
                                 start=True, stop=True)
                px = pexp_pool.tile([P, CH], BF16, tag="pexp")
                nc.scalar.activation(px[:, qoff:], st[:, qoff:], EXP,
                                     scale=SM)
                if P * a >= CH * qt:
                    nc.vector.tensor_mul(out=px[:, qoff:qoff + P],
                                         in0=px[:, qoff:qoff + P],
                                         in1=mask[:])
                pend.append((a, qoff, px))
                if len(pend) == 3:
                    flush()
                yield
            while pend:
                flush()
            rden = rden_pool.tile([P, CH], F32, tag="rden")
            nc.vector.reciprocal_approx_fast(out=rden[:], in_=den_ps[:])
            nc.vector.tensor_mul(
                out=ATn[h][:, S * b + CH * qt:S * b + CH * (qt + 1)],
                in0=at_ps[:], in1=rden[:])
            yield

        def attn_seq(b, qts):
            for qt in qts:
                for h in range(NHL):
                    yield from attn_group(b, h, qt)

        with tc.tile_pool(name="pp", bufs=2, space="PSUM") as pp:
            # chunk 0 alone (nothing else is ready); remaining weight
            # blocks stream in behind its hidden-state loads
            # with bufs=32 the next chunk's xt[j] reuses this chunk's
            # xt[j] (freed progressively through pass 3), so prefetching at
            # the first pass boundary streams supply one full pass ahead
            for _ in proj_chunk(0, pp, side=[prefetch(1)]):
                pass
            # chunk 1 (192y) x b0 qt0 attention (20y)
            interleave(proj_chunk(1, pp, side=[prefetch(2), wo_side[0],
                                               wo_side[1]]),
                       attn_seq(0, [0]), 9)
            # chunks 2+3 (384y) x b0 qt1 attention (36y)
            interleave(chain(proj_chunk(2, pp, side=[prefetch(3),
                                                     wo_side[2],
                                                     wo_side[3]]),
                             proj_chunk(3, pp)),
                       attn_seq(0, [1]), 10)
        xt_stack.close()

        with (
            tc.tile_pool(name="ob", bufs=2) as ob_pool,
            tc.tile_pool(name="opp", bufs=2, space="PSUM") as opp,
        ):
            def oproj_block(t16):
                ob = ob_pool.tile([P, HD], BF16, tag="ob")
                for ot in range(HD // CH):
                    ps = opp.tile([P, CH], F32, tag="op")
                    for j in range(NHL):
                        nc.tensor.matmul(ps[:],
                                         ATn[j][:, P * t16:P * (t16 + 1)],
                                         wo_sb[:, j, CH * ot:CH * (ot + 1)],
                                         start=(j == 0),
                                         stop=(j == NHL - 1))
                        yield
                    nc.any.tensor_copy(ob[:, CH * ot:CH * (ot + 1)], ps[:])
                    if ot % 2 == 1:
                        # write out per 1024-col pair so the final block's
                        # store overlaps its own compute (shorter tail)
                        nc.sync.dma_start(
                            out[P * t16:P * (t16 + 1),
                                CH * (ot - 1):CH * (ot + 1)],
                            ob[:, CH * (ot - 1):CH * (ot + 1)])

            # b1 attention x o_proj; blocks 0-7 (b0) ready at entry,
            # 8-11 after b1 qt0 evacs (attn yield 20), 12-15 at the end
            og = chain(*[oproj_block(t16) for t16 in range(T // P)])
            consumed = 0
            ready = 8
            ay = 0
            for _ in attn_seq(1, [0, 1]):
                ay += 1
                if ay >= 20:
                    ready = max(ready, 12)
                cap = ready * 32
                pulled = 0
                while consumed < cap and pulled < 8:
                    if next(og, _SENT) is _SENT:
                        break
                    consumed += 1
                    pulled += 1
            while next(og, _SENT) is not _SENT:
                pass

    nc.compile()
    return nc


_NC = None


def _get_nc():
    global _NC
    if _NC is None:
        _NC = build_nc()
    return _NC


def make_in_maps(hidden_states, cos, sin, wq, wk, wv, wo):
    bf = ml_dtypes.bfloat16
    hs = np.asarray(hidden_states, np.float32)
    HT = np.ascontiguousarray(hs.T).astype(bf)
    cosT = np.asarray(cos, np.float32).T
    sinT = np.asarray(sin, np.float32).T
    cosF = np.ascontiguousarray(np.concatenate([cosT, cosT], 0)).astype(bf)
    sinF = np.ascontiguousarray(np.concatenate([sinT, -sinT], 0)).astype(bf)
    wq = np.asarray(wq, np.float32)
    wk = np.asarray(wk, np.float32)
    wv = np.asarray(wv, np.float32)
    wo = np.asarray(wo, np.float32)

    def sb_img(Wb):
        # [p, g, c] SBUF image: sb[p, g*128+c] = Wb[c, g*128+p]
        A = np.ascontiguousarray(Wb.T).reshape(KO, P, P)
        return A.transpose(1, 0, 2).reshape(P, KO * P)

    in_maps = []
    for c in range(N_CORES):
        wq_c = wq[NHL * P * c:NHL * P * (c + 1)]
        wk_c = wk[P * c:P * (c + 1)]
        wv_c = wv[P * c:P * (c + 1)]
        blocks = [wk_c] + [wq_c[P * j:P * (j + 1)] for j in range(NHL)] \
            + [wv_c]
        wqB = np.ascontiguousarray(
            np.stack([sb_img(b) for b in blocks], 0)).astype(bf)
        woT = np.ascontiguousarray(
            wo[:, NHL * P * c:NHL * P * (c + 1)].T).astype(bf)
        in_maps.append(dict(hiddenT=HT, wqB=wqB, woT=woT,
                            cosF=cosF, sinF=sinF))
    return in_maps


def kernel(hidden_states, cos, sin, wq, wk, wv, wo, batch, seq_len):
    assert int(batch) == B and int(seq_len) == S
    nc = _get_nc()
    in_maps = make_in_maps(hidden_states, cos, sin, wq, wk, wv, wo)
    res = run_bass_kernel_spmd(nc, in_maps, core_ids=list(range(N_CORES)))
    acc = res.results[0]["out"].astype(np.float32)
    for c in range(1, N_CORES):
        acc += res.results[c]["out"].astype(np.float32)
    return acc


# revision 30
# speedup vs baseline: 1.0384x; 1.0052x over previous
"""Tensor-parallel FlashLlamaAttention kernel for 8 Trainium2 NeuronCores.

Sharding: each core owns 4 query heads (512 proj dims) and 1 kv head
(128 dims). Per-core device program computes qkv projection (+RoPE),
causal GQA attention and its o_proj partial product; the 8 partial
[2048, 4096] outputs are summed on the host (replaces the all-reduce).

v3: fully-fused single PE stream in bf16.
 - all matmul operands bf16 (1 cyc/row, same as f32r, but half the DMA
   and SBUF traffic); PSUM stays f32; host pre-casts inputs, output is
   written bf16 and summed in f32 on the host.
 - projection chunks, attention groups and o_proj blocks are emitted
   interleaved so the PE never idles across phase boundaries (idle gaps
   also reset the PE DVFS ramp).
 - k and v are projected in each chunk's FIRST pass so the next
   segment's attention unblocks a full pass earlier.
 - RoPE rotate-half runs as a partition-crossing bf16 DMA instead of a
   PE permutation matmul.
 - V is projected directly in [token, dim] layout by using the hidden
   chunk as the matmul stationary, killing the PE transposes.
 - softmax denominator is accumulated broadcast across partitions via a
   ones[128,128] stationary and inverted with the fast approximate
   reciprocal (the exact DVE reciprocal costs 3.3us per row-tile and
   serialized the in-order DVE queue).
"""
import sys

sys.path.insert(0, "/opt/trn_rl_repo")

from contextlib import ExitStack

import numpy as np
import ml_dtypes

import concourse.bass as bass
import concourse.bacc as bacc
import concourse.mybir as mybir
import concourse.tile as tile
from concourse.bass_utils import run_bass_kernel_spmd
from concourse.masks import make_identity

F32 = mybir.dt.float32
BF16 = mybir.dt.bfloat16
EXP = mybir.ActivationFunctionType.Exp

P = 128          # partitions / head dim
T = 2048         # total tokens (B * S)
S = 1024         # seq len per batch
B = 2
HD = 4096        # hidden dim
NHL = 4          # local query heads per core
DQKV = NHL * P + P + P  # 768 local projection dims (4q + k + v)
CH = 512         # token chunk for projection
KO = HD // P     # 32 contraction chunks
SM = float(P) ** -0.5

N_CORES = 8

_SENT = object()


def build_nc():
    nc = bacc.Bacc("TRN2", target_bir_lowering=False, debug=False,
                   num_devices=N_CORES)

    hiddenT = nc.dram_tensor("hiddenT", [HD, T], BF16, kind="ExternalInput").ap()
    # wqB holds the six projection blocks (k, q0..q3, v) already in SBUF
    # image layout [p, g, c] so weight DMAs move 2KB-contiguous lines
    wqB = nc.dram_tensor("wqB", [6, P, KO * P], BF16, kind="ExternalInput").ap()
    woT = nc.dram_tensor("woT", [NHL * P, HD], BF16, kind="ExternalInput").ap()
    cosF = nc.dram_tensor("cosF", [P, T], BF16, kind="ExternalInput").ap()
    sinF = nc.dram_tensor("sinF", [P, T], BF16, kind="ExternalInput").ap()
    out = nc.dram_tensor("out", [T, HD], BF16, kind="ExternalOutput").ap()

    with tile.TileContext(nc) as tc, ExitStack() as stack:
        const = stack.enter_context(tc.tile_pool(name="const", bufs=1))
        ident = const.tile([P, P], F32)
        make_identity(nc, ident[:])
        ones_sb = const.tile([P, P], BF16)
        nc.vector.memset(ones_sb[:], 1.0)
        # causal corner mask: keep (q=f) >= (k=p)
        mask = const.tile([P, P], BF16)
        nc.gpsimd.memset(mask[:], 1.0)
        nc.gpsimd.affine_select(
            out=mask[:], in_=mask[:], compare_op=mybir.AluOpType.is_ge,
            fill=0.0, base=0, pattern=[[1, P]], channel_multiplier=-1)

        w1 = stack.enter_context(tc.tile_pool(name="w1", bufs=1))
        wqkv_sb = w1.tile([P, 6, KO, P], BF16)
        cs = stack.enter_context(tc.tile_pool(name="cs", bufs=1))
        cos_sb = cs.tile([P, T], BF16)
        sin_sb = cs.tile([P, T], BF16)
        qk = stack.enter_context(tc.tile_pool(name="qk", bufs=1))
        qT = [qk.tile([P, T], BF16, tag=f"qT{h}", name=f"qT{h}")
              for h in range(NHL)]
        kT = qk.tile([P, T], BF16, tag="kT", name="kT")
        vpool = stack.enter_context(tc.tile_pool(name="vp", bufs=1))
        v_sb = vpool.tile([P, T // P, P], BF16, tag="v_sb", name="v_sb")
        atn_pool = stack.enter_context(tc.tile_pool(name="atn", bufs=1))
        ATn = [atn_pool.tile([P, T], BF16, tag=f"ATn{h}", name=f"ATn{h}")
               for h in range(NHL)]
        w2 = stack.enter_context(tc.tile_pool(name="w2", bufs=1))
        wo_sb = w2.tile([P, NHL, HD], BF16)
        pexp_pool = stack.enter_context(tc.tile_pool(name="pexp", bufs=6))
        rden_pool = stack.enter_context(tc.tile_pool(name="rden", bufs=3))

        # weight loads on the Act DMA ring in quarter-block granularity,
        # ordered by first use: k/v (chunk pass 1), rope tables, then q0..q3
        def wsub(b, i, eng=None):
            (eng or nc.scalar).dma_start(
                wqkv_sb[:, b, 8 * i:8 * (i + 1), :],
                wqB[b, :, 1024 * i:1024 * (i + 1)])
        for i in range(4):
            wsub(0, i)
            wsub(5, i)
        nc.scalar.dma_start(cos_sb[:], cosF[:])
        nc.scalar.dma_start(sin_sb[:], sinF[:])
        for j in (1, 2, 3, 4):
            for i in range(4):
                wsub(j, i)
        # wo is needed only from segment C; its 8 half-slices are issued at
        # pass boundaries inside chunks 1-3 so they never block the rope DMAs
        woT_r = woT.rearrange("(a p) o -> p a o", p=P)
        wo_side = [
            (lambda g=g: nc.sync.dma_start(wo_sb[:, g, :], woT_r[:, g, :]))
            for g in range(NHL)
        ]

        # attention/transpose PSUM pools live for the whole run
        st_psum = stack.enter_context(
            tc.tile_pool(name="stp", bufs=2, space="PSUM"))
        at_psum = stack.enter_context(
            tc.tile_pool(name="atp", bufs=2, space="PSUM"))
        den_psum = stack.enter_context(
            tc.tile_pool(name="dnp", bufs=2, space="PSUM"))

        xt_stack = ExitStack()
        xt_pool = xt_stack.enter_context(tc.tile_pool(name="xt", bufs=32))
        rot_pool = xt_stack.enter_context(tc.tile_pool(name="rot", bufs=3))
        rt_pool = xt_stack.enter_context(tc.tile_pool(name="rt", bufs=3))
        vt_pool = xt_stack.enter_context(tc.tile_pool(name="vt", bufs=2))
        qr_pool = xt_stack.enter_context(tc.tile_pool(name="qr", bufs=3))

        def load_xt(c):
            ts = slice(CH * c, CH * (c + 1))
            xts = []
            for ho in range(KO):
                t = xt_pool.tile([P, CH], BF16, tag="xt",
                                 name=f"xt{c}_{ho}")
                nc.sync.dma_start(t[:], hiddenT[P * ho:P * (ho + 1), ts])
                xts.append(t)
            return xts

        xt_next = {}

        def prefetch(c):
            return lambda: xt_next.__setitem__(c, load_xt(c))

        def proj_chunk(c, pp, side=(), pre=None):
            """qkv projection + rope for tokens [512c, 512c+512).

            k and v are produced in the FIRST pass so the next segment's
            attention (which needs kT/v_sb before qT) unblocks a full pass
            earlier. `side` DMA thunks are fired at pass boundaries.
            """
            side = list(side)
            ts = slice(CH * c, CH * (c + 1))
            xts = xt_next.pop(c, None) or load_xt(c)
            if pre is not None:
                pre()

            def qk_evac1(idx, ps):
                # dst = ps*cosF + rothalf(ps*sinG); sinG = sinF[rot(d)]
                # so the partition-crossing move runs as a bf16 DMA
                dst = qT[idx] if idx < NHL else kT
                nc.vector.tensor_mul(out=dst[:, ts], in0=ps[:],
                                     in1=cos_sb[:, ts])
                qs = rot_pool.tile([P, CH], BF16, tag="qs")
                nc.vector.tensor_mul(out=qs[:], in0=ps[:],
                                     in1=sin_sb[:, ts])
                rt = rt_pool.tile([P, CH], BF16, tag="rt")
                nc.scalar.dma_start(rt[64:128, :], qs[0:64, :])
                nc.scalar.dma_start(rt[0:64, :], qs[64:128, :])
                return dst, rt

            def qk_evac2(st):
                dst, rt = st
                nc.vector.tensor_add(out=dst[:, ts], in0=dst[:, ts],
                                     in1=rt[:])

            def vt_evac1(psV):
                vt = vt_pool.tile([P, CH], F32, tag="vt")
                nc.vector.tensor_copy(vt[:], psV[:])
                return vt

            def vt_evac2(vt):
                # transpose [dim, tok] -> [tok, dim] on the PE via the den
                # psum pool (idle slot between attention groups)
                trp = den_psum.tile([P, CH], F32, tag="den", name=f"trp{c}")
                for s4 in range(4):
                    nc.tensor.transpose(trp[:, P * s4:P * (s4 + 1)],
                                        vt[:, P * s4:P * (s4 + 1)], ident[:])
                nc.vector.tensor_copy(v_sb[:, 4 * c:4 * (c + 1), :], trp[:])

            def kq_pass(idxs, banks, mid=None):
                # one output group per bank, interleaved per-ho across banks
                for ho in range(KO):
                    if ho == 6 and mid is not None:
                        mid()
                    for bi, ps in zip(idxs, banks):
                        nc.tensor.matmul(ps[:], wqkv_sb[:, bi, ho, :],
                                         xts[ho][:], start=(ho == 0),
                                         stop=(ho == KO - 1))
                        yield
                sts = [qk_evac1(0 if bi == 1 else (NHL if bi == 0 else bi - 1),
                                ps)
                       for bi, ps in zip(idxs, banks) if bi != 5]
                vts = [vt_evac1(ps)
                       for bi, ps in zip(idxs, banks) if bi == 5]
                for st_ in sts:
                    qk_evac2(st_)
                return_vals.append(vts)

            def boundary():
                if side:
                    side.pop(0)()

            def bank(nm):
                return pp.tile([P, CH], F32, tag="pp", name=nm)

            return_vals = []
            # chunk 0 runs before any attention, so its later passes borrow
            # the idle st/at pools: every pass gets fresh banks and the
            # pass-boundary WAR on the previous pass's evac disappears
            if c == 0:
                p2 = [st_psum.tile([P, CH], F32, tag="st", name="c0q0"),
                      st_psum.tile([P, CH], F32, tag="st", name="c0q1")]
                p3 = [at_psum.tile([P, CH], F32, tag="at", name="c0q2"),
                      at_psum.tile([P, CH], F32, tag="at", name="c0q3")]
            else:
                p2 = None
                p3 = None
            # pass 1: k + v (blocks 0, 5)
            yield from kq_pass((0, 5), [bank("ppk"), bank("ppv")])
            vt = return_vals[-1][0]
            boundary()
            # pass 2: q0 + q1 (blocks 1, 2); v transpose slots in mid-pass
            yield from kq_pass((1, 2),
                               p2 or [bank("ppq0"), bank("ppq1")],
                               mid=lambda: vt_evac2(vt))
            boundary()
            # pass 3: q2 + q3 (blocks 3, 4)
            yield from kq_pass((3, 4), p3 or [bank("ppq2"), bank("ppq3")])
            boundary()

        def chain(*gens):
            for g in gens:
                yield from g

        def interleave(pg, ag, ratio):
            """1 attention yield : `ratio` proj yields; drain both"""
            done_p = done_a = False
            while not (done_p and done_a):
                if not done_a and next(ag, _SENT) is _SENT:
                    done_a = True
                for _ in range(ratio if not done_a else 1 << 30):
                    if next(pg, _SENT) is _SENT:
                        done_p = True
                        break
                if done_p and not done_a:
                    for _ in ag:
                        pass
                    done_a = True


        def attn_group(b, h, qt):
            """causal attention for one (batch, head, 512-q-block)"""
            qTb = qT[h][:, S * b:S * (b + 1)]
            kTb = kT[:, S * b:S * (b + 1)]
            at_ps = at_psum.tile([P, CH], F32, tag="at")
            den_ps = den_psum.tile([P, CH], F32, tag="den")
            nk = 4 * qt + 4
            pend = []

            def flush():
                a, qo, px = pend.pop(0)
                nc.tensor.matmul(at_ps[:, qo:], v_sb[:, (S // P) * b + a, :],
                                 px[:, qo:], start=(a == 0),
                                 stop=(a == nk - 1))
                nc.tensor.matmul(den_ps[:, qo:], ones_sb[:], px[:, qo:],
                                 start=(a == 0), stop=(a == nk - 1))

            for a in range(nk):
                qoff = max(0, P * a - CH * qt)
                st = st_psum.tile([P, CH], F32, tag="st")
                nc.tensor.matmul(st[:, qoff:], kTb[:, P * a:P * (a + 1)],
                                 qTb[:, CH * qt + qoff:CH * (qt + 1)],
                                 start=True, stop=True)
                px = pexp_pool.tile([P, CH], BF16, tag="pexp")
                nc.scalar.activation(px[:, qoff:], st[:, qoff:], EXP,
                                     scale=SM)
                if P * a >= CH * qt:
                    nc.vector.tensor_mul(out=px[:, qoff:qoff + P],
                                         in0=px[:, qoff:qoff + P],
                                         in1=mask[:])
                pend.append((a, qoff, px))
                if len(pend) == 3:
                    flush()
                yield
            while pend:
                flush()
            rden = rden_pool.tile([P, CH], F32, tag="rden")
            nc.vector.reciprocal_approx_fast(out=rden[:], in_=den_ps[:])
            nc.vector.tensor_mul(
                out=ATn[h][:, S * b + CH * qt:S * b + CH * (qt + 1)],
                in0=at_ps[:], in1=rden[:])
            yield

        def attn_seq(b, qts):
            for qt in qts:
                for h in range(NHL):
                    yield from attn_group(b, h, qt)

        with tc.tile_pool(name="pp", bufs=2, space="PSUM") as pp:
            # chunk 0 alone (nothing else is ready); remaining weight
            # blocks stream in behind its hidden-state loads
            # with bufs=32 the next chunk's xt[j] reuses this chunk's
            # xt[j] (freed progressively through pass 3), so prefetching at
            # the first pass boundary streams supply one full pass ahead
            for _ in proj_chunk(0, pp, side=[prefetch(1)]):
                pass
            # chunk 1 (192y) x b0 qt0 attention (20y)
            interleave(proj_chunk(1, pp, side=[prefetch(2), wo_side[0],
                                               wo_side[1]]),
                       attn_seq(0, [0]), 9)
            # chunks 2+3 (384y) x b0 qt1 attention (36y)
            interleave(chain(proj_chunk(2, pp, side=[prefetch(3),
                                                     wo_side[2],
                                                     wo_side[3]]),
                             proj_chunk(3, pp)),
                       attn_seq(0, [1]), 10)
        xt_stack.close()

        with (
            tc.tile_pool(name="ob", bufs=2) as ob_pool,
            tc.tile_pool(name="opp", bufs=2, space="PSUM") as opp,
        ):
            def oproj_block(t16):
                ob = ob_pool.tile([P, HD], BF16, tag="ob")
                for ot in range(HD // CH):
                    ps = opp.tile([P, CH], F32, tag="op")
                    for j in range(NHL):
                        nc.tensor.matmul(ps[:],
                                         ATn[j][:, P * t16:P * (t16 + 1)],
                                         wo_sb[:, j, CH * ot:CH * (ot + 1)],
                                         start=(j == 0),
                                         stop=(j == NHL - 1))
                        yield
                    nc.any.tensor_copy(ob[:, CH * ot:CH * (ot + 1)], ps[:])
                    if t16 == T // P - 1:
                        nc.sync.dma_start(
                            out[P * t16:P * (t16 + 1),
                                CH * ot:CH * (ot + 1)],
                            ob[:, CH * ot:CH * (ot + 1)])
                    elif ot % 2 == 1:
                        # write out per 1024-col pair so the final block's
                        # store overlaps its own compute (shorter tail)
                        nc.sync.dma_start(
                            out[P * t16:P * (t16 + 1),
                                CH * (ot - 1):CH * (ot + 1)],
                            ob[:, CH * (ot - 1):CH * (ot + 1)])

            # b1 attention x o_proj; blocks 0-7 (b0) ready at entry,
            # 8-11 after b1 qt0 evacs (attn yield 20), 12-15 at the end
            og = chain(*[oproj_block(t16) for t16 in range(T // P)])
            consumed = 0
            ready = 8
            ay = 0
            for _ in attn_seq(1, [0, 1]):
                ay += 1
                if ay >= 20:
                    ready = max(ready, 12)
                cap = ready * 32
                pulled = 0
                while consumed < cap and pulled < 8:
                    if next(og, _SENT) is _SENT:
                        break
                    consumed += 1
                    pulled += 1
            while next(og, _SENT) is not _SENT:
                pass

    nc.compile()
    return nc


_NC = None


def _get_nc():
    global _NC
    if _NC is None:
        _NC = build_nc()
    return _NC


def make_in_maps(hidden_states, cos, sin, wq, wk, wv, wo):
    bf = ml_dtypes.bfloat16
    hs = np.asarray(hidden_states, np.float32)
    HT = np.ascontiguousarray(hs.T).astype(bf)
    cosT = np.asarray(cos, np.float32).T
    sinT = np.asarray(sin, np.float32).T
    cosF = np.ascontiguousarray(np.concatenate([cosT, cosT], 0)).astype(bf)
    sinF = np.ascontiguousarray(np.concatenate([sinT, -sinT], 0)).astype(bf)
    wq = np.asarray(wq, np.float32)
    wk = np.asarray(wk, np.float32)
    wv = np.asarray(wv, np.float32)
    wo = np.asarray(wo, np.float32)

    def sb_img(Wb):
        # [p, g, c] SBUF image: sb[p, g*128+c] = Wb[c, g*128+p]
        A = np.ascontiguousarray(Wb.T).reshape(KO, P, P)
        return A.transpose(1, 0, 2).reshape(P, KO * P)

    in_maps = []
    for c in range(N_CORES):
        wq_c = wq[NHL * P * c:NHL * P * (c + 1)]
        wk_c = wk[P * c:P * (c + 1)]
        wv_c = wv[P * c:P * (c + 1)]
        blocks = [wk_c] + [wq_c[P * j:P * (j + 1)] for j in range(NHL)] \
            + [wv_c]
        wqB = np.ascontiguousarray(
            np.stack([sb_img(b) for b in blocks], 0)).astype(bf)
        woT = np.ascontiguousarray(
            wo[:, NHL * P * c:NHL * P * (c + 1)].T).astype(bf)
        in_maps.append(dict(hiddenT=HT, wqB=wqB, woT=woT,
                            cosF=cosF, sinF=sinF))
    return in_maps


def kernel(hidden_states, cos, sin, wq, wk, wv, wo, batch, seq_len):
    assert int(batch) == B and int(seq_len) == S
    nc = _get_nc()
    in_maps = make_in_maps(hidden_states, cos, sin, wq, wk, wv, wo)
    res = run_bass_kernel_spmd(nc, in_maps, core_ids=list(range(N_CORES)))
    acc = res.results[0]["out"].astype(np.float32)
    for c in range(1, N_CORES):
        acc += res.results[c]["out"].astype(np.float32)
    return acc


# revision 31
# speedup vs baseline: 1.0433x; 1.0047x over previous
"""Tensor-parallel FlashLlamaAttention kernel for 8 Trainium2 NeuronCores.

Sharding: each core owns 4 query heads (512 proj dims) and 1 kv head
(128 dims). Per-core device program computes qkv projection (+RoPE),
causal GQA attention and its o_proj partial product; the 8 partial
[2048, 4096] outputs are summed on the host (replaces the all-reduce).

v3: fully-fused single PE stream in bf16.
 - all matmul operands bf16 (1 cyc/row, same as f32r, but half the DMA
   and SBUF traffic); PSUM stays f32; host pre-casts inputs, output is
   written bf16 and summed in f32 on the host.
 - projection chunks, attention groups and o_proj blocks are emitted
   interleaved so the PE never idles across phase boundaries (idle gaps
   also reset the PE DVFS ramp).
 - k and v are projected in each chunk's FIRST pass so the next
   segment's attention unblocks a full pass earlier.
 - RoPE rotate-half runs as a partition-crossing bf16 DMA instead of a
   PE permutation matmul.
 - V is projected directly in [token, dim] layout by using the hidden
   chunk as the matmul stationary, killing the PE transposes.
 - softmax denominator is accumulated broadcast across partitions via a
   ones[128,128] stationary and inverted with the fast approximate
   reciprocal (the exact DVE reciprocal costs 3.3us per row-tile and
   serialized the in-order DVE queue).
"""
import sys

sys.path.insert(0, "/opt/trn_rl_repo")

from contextlib import ExitStack

import numpy as np
import ml_dtypes

import concourse.bass as bass
import concourse.bacc as bacc
import concourse.mybir as mybir
import concourse.tile as tile
from concourse.bass_utils import run_bass_kernel_spmd
from concourse.masks import make_identity

F32 = mybir.dt.float32
BF16 = mybir.dt.bfloat16
EXP = mybir.ActivationFunctionType.Exp

P = 128          # partitions / head dim
T = 2048         # total tokens (B * S)
S = 1024         # seq len per batch
B = 2
HD = 4096        # hidden dim
NHL = 4          # local query heads per core
DQKV = NHL * P + P + P  # 768 local projection dims (4q + k + v)
CH = 512         # token chunk for projection
KO = HD // P     # 32 contraction chunks
SM = float(P) ** -0.5

N_CORES = 8

_SENT = object()


def build_nc():
    nc = bacc.Bacc("TRN2", target_bir_lowering=False, debug=False,
                   num_devices=N_CORES)

    hiddenT = nc.dram_tensor("hiddenT", [HD, T], BF16, kind="ExternalInput").ap()
    # wqB holds the six projection blocks (k, q0..q3, v) already in SBUF
    # image layout [p, g, c] so weight DMAs move 2KB-contiguous lines
    wqB = nc.dram_tensor("wqB", [6, P, KO * P], BF16, kind="ExternalInput").ap()
    woT = nc.dram_tensor("woT", [NHL * P, HD], BF16, kind="ExternalInput").ap()
    cosF = nc.dram_tensor("cosF", [P, T], BF16, kind="ExternalInput").ap()
    sinF = nc.dram_tensor("sinF", [P, T], BF16, kind="ExternalInput").ap()
    out = nc.dram_tensor("out", [T, HD], BF16, kind="ExternalOutput").ap()

    with tile.TileContext(nc) as tc, ExitStack() as stack:
        const = stack.enter_context(tc.tile_pool(name="const", bufs=1))
        ident = const.tile([P, P], F32)
        make_identity(nc, ident[:])
        ones_sb = const.tile([P, P], BF16)
        nc.vector.memset(ones_sb[:], 1.0)
        # causal corner mask: keep (q=f) >= (k=p)
        mask = const.tile([P, P], BF16)
        nc.gpsimd.memset(mask[:], 1.0)
        nc.gpsimd.affine_select(
            out=mask[:], in_=mask[:], compare_op=mybir.AluOpType.is_ge,
            fill=0.0, base=0, pattern=[[1, P]], channel_multiplier=-1)

        w1 = stack.enter_context(tc.tile_pool(name="w1", bufs=1))
        wqkv_sb = w1.tile([P, 6, KO, P], BF16)
        cs = stack.enter_context(tc.tile_pool(name="cs", bufs=1))
        cos_sb = cs.tile([P, T], BF16)
        sin_sb = cs.tile([P, T], BF16)
        qk = stack.enter_context(tc.tile_pool(name="qk", bufs=1))
        qT = [qk.tile([P, T], BF16, tag=f"qT{h}", name=f"qT{h}")
              for h in range(NHL)]
        kT = qk.tile([P, T], BF16, tag="kT", name="kT")
        vpool = stack.enter_context(tc.tile_pool(name="vp", bufs=1))
        v_sb = vpool.tile([P, T // P, P], BF16, tag="v_sb", name="v_sb")
        atn_pool = stack.enter_context(tc.tile_pool(name="atn", bufs=1))
        ATn = [atn_pool.tile([P, T], BF16, tag=f"ATn{h}", name=f"ATn{h}")
               for h in range(NHL)]
        w2 = stack.enter_context(tc.tile_pool(name="w2", bufs=1))
        wo_sb = w2.tile([P, NHL, HD], BF16)
        pexp_pool = stack.enter_context(tc.tile_pool(name="pexp", bufs=6))
        acc_pool = stack.enter_context(tc.tile_pool(name="acc", bufs=3))
        rden_pool = stack.enter_context(tc.tile_pool(name="rden", bufs=3))

        # weight loads on the Act DMA ring in quarter-block granularity,
        # ordered by first use: k/v (chunk pass 1), rope tables, then q0..q3
        def wsub(b, i, eng=None):
            (eng or nc.scalar).dma_start(
                wqkv_sb[:, b, 8 * i:8 * (i + 1), :],
                wqB[b, :, 1024 * i:1024 * (i + 1)])
        for i in range(4):
            wsub(0, i)
            wsub(5, i)
        nc.scalar.dma_start(cos_sb[:], cosF[:])
        nc.scalar.dma_start(sin_sb[:], sinF[:])
        for j in (1, 2, 3, 4):
            for i in range(4):
                wsub(j, i)
        # wo is needed only from segment C; its 8 half-slices are issued at
        # pass boundaries inside chunks 1-3 so they never block the rope DMAs
        woT_r = woT.rearrange("(a p) o -> p a o", p=P)
        wo_side = [
            (lambda g=g: nc.sync.dma_start(wo_sb[:, g, :], woT_r[:, g, :]))
            for g in range(NHL)
        ]

        # attention/transpose PSUM pools live for the whole run
        st_psum = stack.enter_context(
            tc.tile_pool(name="stp", bufs=2, space="PSUM"))
        at_psum = stack.enter_context(
            tc.tile_pool(name="atp", bufs=2, space="PSUM"))
        den_psum = stack.enter_context(
            tc.tile_pool(name="dnp", bufs=2, space="PSUM"))

        xt_stack = ExitStack()
        xt_pool = xt_stack.enter_context(tc.tile_pool(name="xt", bufs=32))
        rot_pool = xt_stack.enter_context(tc.tile_pool(name="rot", bufs=3))
        rt_pool = xt_stack.enter_context(tc.tile_pool(name="rt", bufs=3))
        vt_pool = xt_stack.enter_context(tc.tile_pool(name="vt", bufs=2))
        qr_pool = xt_stack.enter_context(tc.tile_pool(name="qr", bufs=3))

        def load_xt(c):
            ts = slice(CH * c, CH * (c + 1))
            xts = []
            for ho in range(KO):
                t = xt_pool.tile([P, CH], BF16, tag="xt",
                                 name=f"xt{c}_{ho}")
                nc.sync.dma_start(t[:], hiddenT[P * ho:P * (ho + 1), ts])
                xts.append(t)
            return xts

        xt_next = {}

        def prefetch(c):
            return lambda: xt_next.__setitem__(c, load_xt(c))

        def proj_chunk(c, pp, side=(), pre=None):
            """qkv projection + rope for tokens [512c, 512c+512).

            k and v are produced in the FIRST pass so the next segment's
            attention (which needs kT/v_sb before qT) unblocks a full pass
            earlier. `side` DMA thunks are fired at pass boundaries.
            """
            side = list(side)
            ts = slice(CH * c, CH * (c + 1))
            xts = xt_next.pop(c, None) or load_xt(c)
            if pre is not None:
                pre()

            def qk_evac1(idx, ps):
                # dst = ps*cosF + rothalf(ps*sinG); sinG = sinF[rot(d)]
                # so the partition-crossing move runs as a bf16 DMA
                dst = qT[idx] if idx < NHL else kT
                nc.vector.tensor_mul(out=dst[:, ts], in0=ps[:],
                                     in1=cos_sb[:, ts])
                qs = rot_pool.tile([P, CH], BF16, tag="qs")
                nc.vector.tensor_mul(out=qs[:], in0=ps[:],
                                     in1=sin_sb[:, ts])
                rt = rt_pool.tile([P, CH], BF16, tag="rt")
                nc.scalar.dma_start(rt[64:128, :], qs[0:64, :])
                nc.scalar.dma_start(rt[0:64, :], qs[64:128, :])
                return dst, rt

            def qk_evac2(st):
                dst, rt = st
                nc.vector.tensor_add(out=dst[:, ts], in0=dst[:, ts],
                                     in1=rt[:])

            def vt_evac1(psV):
                vt = vt_pool.tile([P, CH], F32, tag="vt")
                nc.vector.tensor_copy(vt[:], psV[:])
                return vt

            def vt_evac2(vt):
                # transpose [dim, tok] -> [tok, dim] on the PE via the den
                # psum pool (idle slot between attention groups)
                trp = den_psum.tile([P, CH], F32, tag="den", name=f"trp{c}")
                for s4 in range(4):
                    nc.tensor.transpose(trp[:, P * s4:P * (s4 + 1)],
                                        vt[:, P * s4:P * (s4 + 1)], ident[:])
                nc.vector.tensor_copy(v_sb[:, 4 * c:4 * (c + 1), :], trp[:])

            def kq_pass(idxs, banks, mid=None):
                # one output group per bank, interleaved per-ho across banks
                for ho in range(KO):
                    if ho == 6 and mid is not None:
                        mid()
                    for bi, ps in zip(idxs, banks):
                        nc.tensor.matmul(ps[:], wqkv_sb[:, bi, ho, :],
                                         xts[ho][:], start=(ho == 0),
                                         stop=(ho == KO - 1))
                        yield
                sts = [qk_evac1(0 if bi == 1 else (NHL if bi == 0 else bi - 1),
                                ps)
                       for bi, ps in zip(idxs, banks) if bi != 5]
                vts = [vt_evac1(ps)
                       for bi, ps in zip(idxs, banks) if bi == 5]
                for st_ in sts:
                    qk_evac2(st_)
                return_vals.append(vts)

            def boundary():
                if side:
                    side.pop(0)()

            def bank(nm):
                return pp.tile([P, CH], F32, tag="pp", name=nm)

            return_vals = []
            # chunk 0 runs before any attention, so its later passes borrow
            # the idle st/at pools: every pass gets fresh banks and the
            # pass-boundary WAR on the previous pass's evac disappears
            if c == 0:
                p2 = [st_psum.tile([P, CH], F32, tag="st", name="c0q0"),
                      st_psum.tile([P, CH], F32, tag="st", name="c0q1")]
                p3 = [at_psum.tile([P, CH], F32, tag="at", name="c0q2"),
                      at_psum.tile([P, CH], F32, tag="at", name="c0q3")]
            else:
                p2 = None
                p3 = None
            # pass 1: k + v (blocks 0, 5)
            yield from kq_pass((0, 5), [bank("ppk"), bank("ppv")])
            vt = return_vals[-1][0]
            boundary()
            # pass 2: q0 + q1 (blocks 1, 2); v transpose slots in mid-pass
            yield from kq_pass((1, 2),
                               p2 or [bank("ppq0"), bank("ppq1")],
                               mid=lambda: vt_evac2(vt))
            boundary()
            # pass 3: q2 + q3 (blocks 3, 4)
            yield from kq_pass((3, 4), p3 or [bank("ppq2"), bank("ppq3")])
            boundary()

        def chain(*gens):
            for g in gens:
                yield from g

        def interleave(pg, ag, ratio):
            """1 attention yield : `ratio` proj yields; drain both"""
            done_p = done_a = False
            while not (done_p and done_a):
                if not done_a and next(ag, _SENT) is _SENT:
                    done_a = True
                for _ in range(ratio if not done_a else 1 << 30):
                    if next(pg, _SENT) is _SENT:
                        done_p = True
                        break
                if done_p and not done_a:
                    for _ in ag:
                        pass
                    done_a = True


        def attn_group(b, h, qt):
            """causal attention for one (batch, head, 512-q-block)"""
            qTb = qT[h][:, S * b:S * (b + 1)]
            kTb = kT[:, S * b:S * (b + 1)]
            at_ps = at_psum.tile([P, CH], F32, tag="at")
            den_ps = den_psum.tile([P, CH], F32, tag="den")
            acc = acc_pool.tile([P, CH], BF16, tag="acc")
            nk = 4 * qt + 4
            pend = []

            def flush():
                # pexp tiles are summed on DVE (2X bf16); one 512-row den
                # matmul per group replaces nk per-tile ones-matmuls
                a, qo, px = pend.pop(0)
                nc.tensor.matmul(at_ps[:, qo:], v_sb[:, (S // P) * b + a, :],
                                 px[:, qo:], start=(a == 0),
                                 stop=(a == nk - 1))
                if a == 0:
                    nc.vector.tensor_copy(acc[:], px[:])
                else:
                    nc.vector.tensor_add(out=acc[:, qo:], in0=acc[:, qo:],
                                         in1=px[:, qo:])
                if a == nk - 1:
                    nc.tensor.matmul(den_ps[:], ones_sb[:], acc[:],
                                     start=True, stop=True)

            for a in range(nk):
                qoff = max(0, P * a - CH * qt)
                st = st_psum.tile([P, CH], F32, tag="st")
                nc.tensor.matmul(st[:, qoff:], kTb[:, P * a:P * (a + 1)],
                                 qTb[:, CH * qt + qoff:CH * (qt + 1)],
                                 start=True, stop=True)
                px = pexp_pool.tile([P, CH], BF16, tag="pexp")
                nc.scalar.activation(px[:, qoff:], st[:, qoff:], EXP,
                                     scale=SM)
                if P * a >= CH * qt:
                    nc.vector.tensor_mul(out=px[:, qoff:qoff + P],
                                         in0=px[:, qoff:qoff + P],
                                         in1=mask[:])
                pend.append((a, qoff, px))
                if len(pend) == 3:
                    flush()
                yield
            while pend:
                flush()
            rden = rden_pool.tile([P, CH], F32, tag="rden")
            nc.vector.reciprocal_approx_fast(out=rden[:], in_=den_ps[:])
            nc.vector.tensor_mul(
                out=ATn[h][:, S * b + CH * qt:S * b + CH * (qt + 1)],
                in0=at_ps[:], in1=rden[:])
            yield

        def attn_seq(b, qts):
            for qt in qts:
                for h in range(NHL):
                    yield from attn_group(b, h, qt)

        with tc.tile_pool(name="pp", bufs=2, space="PSUM") as pp:
            # chunk 0 alone (nothing else is ready); remaining weight
            # blocks stream in behind its hidden-state loads
            # with bufs=32 the next chunk's xt[j] reuses this chunk's
            # xt[j] (freed progressively through pass 3), so prefetching at
            # the first pass boundary streams supply one full pass ahead
            for _ in proj_chunk(0, pp, side=[prefetch(1)]):
                pass
            # chunk 1 (192y) x b0 qt0 attention (20y)
            interleave(proj_chunk(1, pp, side=[prefetch(2), wo_side[0],
                                               wo_side[1]]),
                       attn_seq(0, [0]), 9)
            # chunks 2+3 (384y) x b0 qt1 attention (36y)
            interleave(chain(proj_chunk(2, pp, side=[prefetch(3),
                                                     wo_side[2],
                                                     wo_side[3]]),
                             proj_chunk(3, pp)),
                       attn_seq(0, [1]), 10)
        xt_stack.close()

        with (
            tc.tile_pool(name="ob", bufs=2) as ob_pool,
            tc.tile_pool(name="opp", bufs=2, space="PSUM") as opp,
        ):
            def oproj_block(t16):
                ob = ob_pool.tile([P, HD], BF16, tag="ob")
                for ot in range(HD // CH):
                    ps = opp.tile([P, CH], F32, tag="op")
                    for j in range(NHL):
                        nc.tensor.matmul(ps[:],
                                         ATn[j][:, P * t16:P * (t16 + 1)],
                                         wo_sb[:, j, CH * ot:CH * (ot + 1)],
                                         start=(j == 0),
                                         stop=(j == NHL - 1))
                        yield
                    nc.any.tensor_copy(ob[:, CH * ot:CH * (ot + 1)], ps[:])
                    if t16 == T // P - 1:
                        nc.sync.dma_start(
                            out[P * t16:P * (t16 + 1),
                                CH * ot:CH * (ot + 1)],
                            ob[:, CH * ot:CH * (ot + 1)])
                    elif ot % 2 == 1:
                        # write out per 1024-col pair so the final block's
                        # store overlaps its own compute (shorter tail)
                        nc.sync.dma_start(
                            out[P * t16:P * (t16 + 1),
                                CH * (ot - 1):CH * (ot + 1)],
                            ob[:, CH * (ot - 1):CH * (ot + 1)])

            # b1 attention x o_proj; blocks 0-7 (b0) ready at entry,
            # 8-11 after b1 qt0 evacs (attn yield 20), 12-15 at the end
            og = chain(*[oproj_block(t16) for t16 in range(T // P)])
            consumed = 0
            ready = 8
            ay = 0
            for _ in attn_seq(1, [0, 1]):
                ay += 1
                if ay >= 20:
                    ready = max(ready, 12)
                cap = ready * 32
                pulled = 0
                while consumed < cap and pulled < 8:
                    if next(og, _SENT) is _SENT:
                        break
                    consumed += 1
                    pulled += 1
            while next(og, _SENT) is not _SENT:
                pass

    nc.compile()
    return nc


_NC = None


def _get_nc():
    global _NC
    if _NC is None:
        _NC = build_nc()
    return _NC


def make_in_maps(hidden_states, cos, sin, wq, wk, wv, wo):
    bf = ml_dtypes.bfloat16
    hs = np.asarray(hidden_states, np.float32)
    HT = np.ascontiguousarray(hs.T).astype(bf)
    cosT = np.asarray(cos, np.float32).T
    sinT = np.asarray(sin, np.float32).T
    cosF = np.ascontiguousarray(np.concatenate([cosT, cosT], 0)).astype(bf)
    sinF = np.ascontiguousarray(np.concatenate([sinT, -sinT], 0)).astype(bf)
    wq = np.asarray(wq, np.float32)
    wk = np.asarray(wk, np.float32)
    wv = np.asarray(wv, np.float32)
    wo = np.asarray(wo, np.float32)

    def sb_img(Wb):
        # [p, g, c] SBUF image: sb[p, g*128+c] = Wb[c, g*128+p]
        A = np.ascontiguousarray(Wb.T).reshape(KO, P, P)
        return A.transpose(1, 0, 2).reshape(P, KO * P)

    in_maps = []
    for c in range(N_CORES):
        wq_c = wq[NHL * P * c:NHL * P * (c + 1)]
        wk_c = wk[P * c:P * (c + 1)]
        wv_c = wv[P * c:P * (c + 1)]
        blocks = [wk_c] + [wq_c[P * j:P * (j + 1)] for j in range(NHL)] \
            + [wv_c]
        wqB = np.ascontiguousarray(
            np.stack([sb_img(b) for b in blocks], 0)).astype(bf)
        woT = np.ascontiguousarray(
            wo[:, NHL * P * c:NHL * P * (c + 1)].T).astype(bf)
        in_maps.append(dict(hiddenT=HT, wqB=wqB, woT=woT,
                            cosF=cosF, sinF=sinF))
    return in_maps


def kernel(hidden_states, cos, sin, wq, wk, wv, wo, batch, seq_len):
    assert int(batch) == B and int(seq_len) == S
    nc = _get_nc()
    in_maps = make_in_maps(hidden_states, cos, sin, wq, wk, wv, wo)
    res = run_bass_kernel_spmd(nc, in_maps, core_ids=list(range(N_CORES)))
    acc = res.results[0]["out"].astype(np.float32)
    for c in range(1, N_CORES):
        acc += res.results[c]["out"].astype(np.float32)
    return acc


# revision 32
# speedup vs baseline: 1.0697x; 1.0254x over previous
"""Tensor-parallel FlashLlamaAttention kernel for 8 Trainium2 NeuronCores.

Sharding: each core owns 4 query heads (512 proj dims) and 1 kv head
(128 dims). Per-core device program computes qkv projection (+RoPE),
causal GQA attention and its o_proj partial product; the 8 partial
[2048, 4096] outputs are summed on the host (replaces the all-reduce).

v3: fully-fused single PE stream in bf16.
 - all matmul operands bf16 (1 cyc/row, same as f32r, but half the DMA
   and SBUF traffic); PSUM stays f32; host pre-casts inputs, output is
   written bf16 and summed in f32 on the host.
 - projection chunks, attention groups and o_proj blocks are emitted
   interleaved so the PE never idles across phase boundaries (idle gaps
   also reset the PE DVFS ramp).
 - k and v are projected in each chunk's FIRST pass so the next
   segment's attention unblocks a full pass earlier.
 - RoPE rotate-half runs as a partition-crossing bf16 DMA instead of a
   PE permutation matmul.
 - V is projected directly in [token, dim] layout by using the hidden
   chunk as the matmul stationary, killing the PE transposes.
 - softmax denominator is accumulated broadcast across partitions via a
   ones[128,128] stationary and inverted with the fast approximate
   reciprocal (the exact DVE reciprocal costs 3.3us per row-tile and
   serialized the in-order DVE queue).
"""
import sys

sys.path.insert(0, "/opt/trn_rl_repo")

from contextlib import ExitStack

import numpy as np
import ml_dtypes

import concourse.bass as bass
import concourse.bacc as bacc
import concourse.mybir as mybir
import concourse.tile as tile
from concourse.bass_utils import run_bass_kernel_spmd
from concourse.masks import make_identity

F32 = mybir.dt.float32
BF16 = mybir.dt.bfloat16
EXP = mybir.ActivationFunctionType.Exp

P = 128          # partitions / head dim
T = 2048         # total tokens (B * S)
S = 1024         # seq len per batch
B = 2
HD = 4096        # hidden dim
NHL = 4          # local query heads per core
DQKV = NHL * P + P + P  # 768 local projection dims (4q + k + v)
CH = 512         # token chunk for projection
KO = HD // P     # 32 contraction chunks
SM = float(P) ** -0.5

N_CORES = 8

_SENT = object()


def build_nc():
    nc = bacc.Bacc("TRN2", target_bir_lowering=False, debug=False,
                   num_devices=N_CORES)

    hiddenT = nc.dram_tensor("hiddenT", [HD, T], BF16, kind="ExternalInput").ap()
    # wqB holds the six projection blocks (k, q0..q3, v) already in SBUF
    # image layout [p, g, c] so weight DMAs move 2KB-contiguous lines
    wqB = nc.dram_tensor("wqB", [6, P, KO * P], BF16, kind="ExternalInput").ap()
    woT = nc.dram_tensor("woT", [NHL * P, HD], BF16, kind="ExternalInput").ap()
    cosF = nc.dram_tensor("cosF", [P, T], BF16, kind="ExternalInput").ap()
    sinF = nc.dram_tensor("sinF", [P, T], BF16, kind="ExternalInput").ap()
    out = nc.dram_tensor("out", [T, HD], BF16, kind="ExternalOutput").ap()

    with tile.TileContext(nc) as tc, ExitStack() as stack:
        const = stack.enter_context(tc.tile_pool(name="const", bufs=1))
        ident = const.tile([P, P], F32)
        make_identity(nc, ident[:])
        ones_sb = const.tile([P, P], BF16)
        nc.vector.memset(ones_sb[:], 1.0)
        # causal corner mask: keep (q=f) >= (k=p)
        mask = const.tile([P, P], BF16)
        nc.gpsimd.memset(mask[:], 1.0)
        nc.gpsimd.affine_select(
            out=mask[:], in_=mask[:], compare_op=mybir.AluOpType.is_ge,
            fill=0.0, base=0, pattern=[[1, P]], channel_multiplier=-1)

        w1 = stack.enter_context(tc.tile_pool(name="w1", bufs=1))
        wqkv_sb = w1.tile([P, 6, KO, P], BF16)
        cs = stack.enter_context(tc.tile_pool(name="cs", bufs=1))
        cos_sb = cs.tile([P, T], BF16)
        sin_sb = cs.tile([P, T], BF16)
        qk = stack.enter_context(tc.tile_pool(name="qk", bufs=1))
        qT = [qk.tile([P, T], BF16, tag=f"qT{h}", name=f"qT{h}")
              for h in range(NHL)]
        kT = qk.tile([P, T], BF16, tag="kT", name="kT")
        vpool = stack.enter_context(tc.tile_pool(name="vp", bufs=1))
        v_sb = vpool.tile([P, T // P, P], BF16, tag="v_sb", name="v_sb")
        atn_pool = stack.enter_context(tc.tile_pool(name="atn", bufs=1))
        ATn = [atn_pool.tile([P, T], BF16, tag=f"ATn{h}", name=f"ATn{h}")
               for h in range(NHL)]
        w2 = stack.enter_context(tc.tile_pool(name="w2", bufs=1))
        wo_sb = w2.tile([P, NHL, HD], BF16)
        pexp_pool = stack.enter_context(tc.tile_pool(name="pexp", bufs=8))
        acc_pool = stack.enter_context(tc.tile_pool(name="acc", bufs=3))
        rden_pool = stack.enter_context(tc.tile_pool(name="rden", bufs=3))

        # weight loads on the Act DMA ring in quarter-block granularity,
        # ordered by first use: k/v (chunk pass 1), rope tables, then q0..q3
        def wsub(b, i, eng=None):
            (eng or nc.scalar).dma_start(
                wqkv_sb[:, b, 8 * i:8 * (i + 1), :],
                wqB[b, :, 1024 * i:1024 * (i + 1)])
        for i in range(4):
            wsub(0, i)
            wsub(5, i)
        nc.scalar.dma_start(cos_sb[:], cosF[:])
        nc.scalar.dma_start(sin_sb[:], sinF[:])
        for j in (1, 2, 3, 4):
            for i in range(4):
                wsub(j, i)
        # wo is needed only from segment C; its 8 half-slices are issued at
        # pass boundaries inside chunks 1-3 so they never block the rope DMAs
        woT_r = woT.rearrange("(a p) o -> p a o", p=P)
        wo_side = [
            (lambda g=g: nc.sync.dma_start(wo_sb[:, g, :], woT_r[:, g, :]))
            for g in range(NHL)
        ]

        # attention/transpose PSUM pools live for the whole run
        # den needs only 1 bank now (one short-lived matmul per group),
        # freeing a bank for a 3rd score buffer (deeper st->exp->AV pipe)
        st_psum = stack.enter_context(
            tc.tile_pool(name="stp", bufs=3, space="PSUM"))
        at_psum = stack.enter_context(
            tc.tile_pool(name="atp", bufs=2, space="PSUM"))
        den_psum = stack.enter_context(
            tc.tile_pool(name="dnp", bufs=1, space="PSUM"))

        xt_stack = ExitStack()
        xt_pool = xt_stack.enter_context(tc.tile_pool(name="xt", bufs=32))
        rot_pool = xt_stack.enter_context(tc.tile_pool(name="rot", bufs=3))
        rt_pool = xt_stack.enter_context(tc.tile_pool(name="rt", bufs=3))
        vt_pool = xt_stack.enter_context(tc.tile_pool(name="vt", bufs=2))
        qr_pool = xt_stack.enter_context(tc.tile_pool(name="qr", bufs=3))

        def load_xt(c):
            ts = slice(CH * c, CH * (c + 1))
            xts = []
            for ho in range(KO):
                t = xt_pool.tile([P, CH], BF16, tag="xt",
                                 name=f"xt{c}_{ho}")
                nc.sync.dma_start(t[:], hiddenT[P * ho:P * (ho + 1), ts])
                xts.append(t)
            return xts

        xt_next = {}

        def prefetch(c):
            return lambda: xt_next.__setitem__(c, load_xt(c))

        def proj_chunk(c, pp, side=(), pre=None):
            """qkv projection + rope for tokens [512c, 512c+512).

            k and v are produced in the FIRST pass so the next segment's
            attention (which needs kT/v_sb before qT) unblocks a full pass
            earlier. `side` DMA thunks are fired at pass boundaries.
            """
            side = list(side)
            ts = slice(CH * c, CH * (c + 1))
            xts = xt_next.pop(c, None) or load_xt(c)
            if pre is not None:
                pre()

            def qk_evac1(idx, ps):
                # dst = ps*cosF + rothalf(ps*sinG); sinG = sinF[rot(d)]
                # so the partition-crossing move runs as a bf16 DMA
                dst = qT[idx] if idx < NHL else kT
                nc.vector.tensor_mul(out=dst[:, ts], in0=ps[:],
                                     in1=cos_sb[:, ts])
                qs = rot_pool.tile([P, CH], BF16, tag="qs")
                nc.vector.tensor_mul(out=qs[:], in0=ps[:],
                                     in1=sin_sb[:, ts])
                rt = rt_pool.tile([P, CH], BF16, tag="rt")
                nc.scalar.dma_start(rt[64:128, :], qs[0:64, :])
                nc.scalar.dma_start(rt[0:64, :], qs[64:128, :])
                return dst, rt

            def qk_evac2(st):
                dst, rt = st
                nc.vector.tensor_add(out=dst[:, ts], in0=dst[:, ts],
                                     in1=rt[:])

            def vt_evac1(psV):
                vt = vt_pool.tile([P, CH], F32, tag="vt")
                nc.vector.tensor_copy(vt[:], psV[:])
                return vt

            def vt_evac2(vt):
                # transpose [dim, tok] -> [tok, dim] on the PE via the den
                # psum pool (idle slot between attention groups)
                trp = den_psum.tile([P, CH], F32, tag="den", name=f"trp{c}")
                for s4 in range(4):
                    nc.tensor.transpose(trp[:, P * s4:P * (s4 + 1)],
                                        vt[:, P * s4:P * (s4 + 1)], ident[:])
                nc.vector.tensor_copy(v_sb[:, 4 * c:4 * (c + 1), :], trp[:])

            def kq_pass(idxs, banks, mid=None):
                # one output group per bank, interleaved per-ho across banks
                for ho in range(KO):
                    if ho == 6 and mid is not None:
                        mid()
                    for bi, ps in zip(idxs, banks):
                        nc.tensor.matmul(ps[:], wqkv_sb[:, bi, ho, :],
                                         xts[ho][:], start=(ho == 0),
                                         stop=(ho == KO - 1))
                        yield
                sts = [qk_evac1(0 if bi == 1 else (NHL if bi == 0 else bi - 1),
                                ps)
                       for bi, ps in zip(idxs, banks) if bi != 5]
                vts = [vt_evac1(ps)
                       for bi, ps in zip(idxs, banks) if bi == 5]
                for st_ in sts:
                    qk_evac2(st_)
                return_vals.append(vts)

            def boundary():
                if side:
                    side.pop(0)()

            def bank(nm):
                return pp.tile([P, CH], F32, tag="pp", name=nm)

            return_vals = []
            # chunk 0 runs before any attention, so its later passes borrow
            # the idle st/at pools: every pass gets fresh banks and the
            # pass-boundary WAR on the previous pass's evac disappears
            if c == 0:
                p2 = [st_psum.tile([P, CH], F32, tag="st", name="c0q0"),
                      st_psum.tile([P, CH], F32, tag="st", name="c0q1")]
                p3 = [at_psum.tile([P, CH], F32, tag="at", name="c0q2"),
                      at_psum.tile([P, CH], F32, tag="at", name="c0q3")]
            else:
                p2 = None
                p3 = None
            # pass 1: k + v (blocks 0, 5)
            yield from kq_pass((0, 5), [bank("ppk"), bank("ppv")])
            vt = return_vals[-1][0]
            boundary()
            # pass 2: q0 + q1 (blocks 1, 2); v transpose slots in mid-pass
            yield from kq_pass((1, 2),
                               p2 or [bank("ppq0"), bank("ppq1")],
                               mid=lambda: vt_evac2(vt))
            boundary()
            # pass 3: q2 + q3 (blocks 3, 4)
            yield from kq_pass((3, 4), p3 or [bank("ppq2"), bank("ppq3")])
            boundary()

        def chain(*gens):
            for g in gens:
                yield from g

        def interleave(pg, ag, ratio):
            """1 attention yield : `ratio` proj yields; drain both"""
            done_p = done_a = False
            while not (done_p and done_a):
                if not done_a and next(ag, _SENT) is _SENT:
                    done_a = True
                for _ in range(ratio if not done_a else 1 << 30):
                    if next(pg, _SENT) is _SENT:
                        done_p = True
                        break
                if done_p and not done_a:
                    for _ in ag:
                        pass
                    done_a = True


        def attn_group(b, h, qt):
            """causal attention for one (batch, head, 512-q-block)"""
            qTb = qT[h][:, S * b:S * (b + 1)]
            kTb = kT[:, S * b:S * (b + 1)]
            at_ps = at_psum.tile([P, CH], F32, tag="at")
            den_ps = den_psum.tile([P, CH], F32, tag="den")
            acc = acc_pool.tile([P, CH], BF16, tag="acc")
            nk = 4 * qt + 4
            pend = []

            def flush():
                # pexp tiles are summed on DVE (2X bf16); one 512-row den
                # matmul per group replaces nk per-tile ones-matmuls
                a, qo, px = pend.pop(0)
                nc.tensor.matmul(at_ps[:, qo:], v_sb[:, (S // P) * b + a, :],
                                 px[:, qo:], start=(a == 0),
                                 stop=(a == nk - 1))
                if a == 0:
                    nc.vector.tensor_copy(acc[:], px[:])
                else:
                    nc.vector.tensor_add(out=acc[:, qo:], in0=acc[:, qo:],
                                         in1=px[:, qo:])
                if a == nk - 1:
                    nc.tensor.matmul(den_ps[:], ones_sb[:], acc[:],
                                     start=True, stop=True)

            for a in range(nk):
                qoff = max(0, P * a - CH * qt)
                st = st_psum.tile([P, CH], F32, tag="st")
                nc.tensor.matmul(st[:, qoff:], kTb[:, P * a:P * (a + 1)],
                                 qTb[:, CH * qt + qoff:CH * (qt + 1)],
                                 start=True, stop=True)
                px = pexp_pool.tile([P, CH], BF16, tag="pexp")
                nc.scalar.activation(px[:, qoff:], st[:, qoff:], EXP,
                                     scale=SM)
                if P * a >= CH * qt:
                    nc.vector.tensor_mul(out=px[:, qoff:qoff + P],
                                         in0=px[:, qoff:qoff + P],
                                         in1=mask[:])
                pend.append((a, qoff, px))
                if len(pend) == 3:
                    flush()
                yield
            while pend:
                flush()
            rden = rden_pool.tile([P, CH], F32, tag="rden")
            nc.vector.reciprocal_approx_fast(out=rden[:], in_=den_ps[:])
            nc.vector.tensor_mul(
                out=ATn[h][:, S * b + CH * qt:S * b + CH * (qt + 1)],
                in0=at_ps[:], in1=rden[:])
            yield

        def attn_seq(b, qts):
            for qt in qts:
                for h in range(NHL):
                    yield from attn_group(b, h, qt)

        with tc.tile_pool(name="pp", bufs=2, space="PSUM") as pp:
            # chunk 0 alone (nothing else is ready); remaining weight
            # blocks stream in behind its hidden-state loads
            # with bufs=32 the next chunk's xt[j] reuses this chunk's
            # xt[j] (freed progressively through pass 3), so prefetching at
            # the first pass boundary streams supply one full pass ahead
            for _ in proj_chunk(0, pp, side=[prefetch(1)]):
                pass
            # chunk 1 (192y) x b0 qt0 attention (20y)
            interleave(proj_chunk(1, pp, side=[prefetch(2), wo_side[0],
                                               wo_side[1]]),
                       attn_seq(0, [0]), 9)
            # chunks 2+3 (384y) x b0 qt1 attention (36y)
            interleave(chain(proj_chunk(2, pp, side=[prefetch(3),
                                                     wo_side[2],
                                                     wo_side[3]]),
                             proj_chunk(3, pp)),
                       attn_seq(0, [1]), 10)
        xt_stack.close()

        with (
            tc.tile_pool(name="ob", bufs=2) as ob_pool,
            tc.tile_pool(name="opp", bufs=2, space="PSUM") as opp,
        ):
            def oproj_block(t16):
                ob = ob_pool.tile([P, HD], BF16, tag="ob")
                for ot in range(HD // CH):
                    ps = opp.tile([P, CH], F32, tag="op")
                    for j in range(NHL):
                        nc.tensor.matmul(ps[:],
                                         ATn[j][:, P * t16:P * (t16 + 1)],
                                         wo_sb[:, j, CH * ot:CH * (ot + 1)],
                                         start=(j == 0),
                                         stop=(j == NHL - 1))
                        yield
                    nc.any.tensor_copy(ob[:, CH * ot:CH * (ot + 1)], ps[:])
                    if t16 == T // P - 1:
                        nc.sync.dma_start(
                            out[P * t16:P * (t16 + 1),
                                CH * ot:CH * (ot + 1)],
                            ob[:, CH * ot:CH * (ot + 1)])
                    elif ot % 2 == 1:
                        # write out per 1024-col pair so the final block's
                        # store overlaps its own compute (shorter tail)
                        nc.sync.dma_start(
                            out[P * t16:P * (t16 + 1),
                                CH * (ot - 1):CH * (ot + 1)],
                            ob[:, CH * (ot - 1):CH * (ot + 1)])

            # b1 attention x o_proj; blocks 0-7 (b0) ready at entry,
            # 8-11 after b1 qt0 evacs (attn yield 20), 12-15 at the end
            og = chain(*[oproj_block(t16) for t16 in range(T // P)])
            consumed = 0
            ready = 8
            ay = 0
            for _ in attn_seq(1, [0, 1]):
                ay += 1
                if ay >= 20:
                    ready = max(ready, 12)
                cap = ready * 32
                pulled = 0
                while consumed < cap and pulled < 8:
                    if next(og, _SENT) is _SENT:
                        break
                    consumed += 1
                    pulled += 1
            while next(og, _SENT) is not _SENT:
                pass

    nc.compile()
    return nc


_NC = None


def _get_nc():
    global _NC
    if _NC is None:
        _NC = build_nc()
    return _NC


def make_in_maps(hidden_states, cos, sin, wq, wk, wv, wo):
    bf = ml_dtypes.bfloat16
    hs = np.asarray(hidden_states, np.float32)
    HT = np.ascontiguousarray(hs.T).astype(bf)
    cosT = np.asarray(cos, np.float32).T
    sinT = np.asarray(sin, np.float32).T
    cosF = np.ascontiguousarray(np.concatenate([cosT, cosT], 0)).astype(bf)
    sinF = np.ascontiguousarray(np.concatenate([sinT, -sinT], 0)).astype(bf)
    wq = np.asarray(wq, np.float32)
    wk = np.asarray(wk, np.float32)
    wv = np.asarray(wv, np.float32)
    wo = np.asarray(wo, np.float32)

    def sb_img(Wb):
        # [p, g, c] SBUF image: sb[p, g*128+c] = Wb[c, g*128+p]
        A = np.ascontiguousarray(Wb.T).reshape(KO, P, P)
        return A.transpose(1, 0, 2).reshape(P, KO * P)

    in_maps = []
    for c in range(N_CORES):
        wq_c = wq[NHL * P * c:NHL * P * (c + 1)]
        wk_c = wk[P * c:P * (c + 1)]
        wv_c = wv[P * c:P * (c + 1)]
        blocks = [wk_c] + [wq_c[P * j:P * (j + 1)] for j in range(NHL)] \
            + [wv_c]
        wqB = np.ascontiguousarray(
            np.stack([sb_img(b) for b in blocks], 0)).astype(bf)
        woT = np.ascontiguousarray(
            wo[:, NHL * P * c:NHL * P * (c + 1)].T).astype(bf)
        in_maps.append(dict(hiddenT=HT, wqB=wqB, woT=woT,
                            cosF=cosF, sinF=sinF))
    return in_maps


def kernel(hidden_states, cos, sin, wq, wk, wv, wo, batch, seq_len):
    assert int(batch) == B and int(seq_len) == S
    nc = _get_nc()
    in_maps = make_in_maps(hidden_states, cos, sin, wq, wk, wv, wo)
    res = run_bass_kernel_spmd(nc, in_maps, core_ids=list(range(N_CORES)))
    acc = res.results[0]["out"].astype(np.float32)
    for c in range(1, N_CORES):
        acc += res.results[c]["out"].astype(np.float32)
    return acc
